# revision 1
# baseline (speedup 1.0000x reference)
"""Trainium2 Bass kernel for nn_Attention_609885356930.

Reference math (per batch b, sequence s):
    term1[b,s,k] = sum_d WO[k,d] * x[b,s,d]          # big matmul
    term2[b,k]   = sum_d WG[k,d] * g[b,d]            # tiny matmul
    out[b,s]     = sum_k v[k] * tanh(term1 + term2)

Strategy (8 NeuronCores, data-parallel over batch, 4 batches/core):
  - Host pre-transposes x -> xT[b, d, s] and weights -> WO.T / WG.T (bf16),
    so the contraction dim d lands on SBUF partitions with no on-device
    transpose.
  - Compute term1 transposed on-chip: T1[k_block, s] so that
      * term2 becomes a per-partition bias fused into the ACT tanh pass
      * the v-weighted reduce over k runs on the otherwise-idle DVE as
        per-partition-scalar multiply-accumulates, finished by a single
        ones-vector PE matmul per s-block (partition reduction).
  - bf16 matmuls (rel-err budget 2e-2), fp32 PSUM accumulation; the final
    k-accumulate rounds to bf16 so the partition-reduce matmul is
    single-pass (fp32 PE matmuls are two-pass LOW/HIGH).
  - Startup: memset-fed dummy matmuls warm the PE (HAM un-throttle) with no
    DMA dependency; weights stream in k-halves so term2 and the first main
    m-groups start after ~3 MB of DMA instead of 5 MB.
"""

import sys
import types
import numpy as np
import ml_dtypes
from contextlib import ExitStack

import concourse.bass as bass
import concourse.mybir as mybir
import concourse.tile as tile
from concourse import bacc
from concourse.bass_utils import run_bass_kernel_spmd


def _ensure_trace_support():
    """Make run_bass_kernel_spmd(trace=True) (or BASS_TRACE=1) work under
    axon even when the image's antenv lacks the axon_hooks module; degrade
    silently if anything is missing."""
    try:
        try:
            from antenv.axon_hooks import get_axon_ntff_profile_hook  # noqa: F401
        except ImportError:
            import antenv
            from trn_agent_boot.trn_boot import _ntff_profile_via_ctypes

            mod = types.ModuleType("antenv.axon_hooks")
            state = {"hook": None}
            mod.set_axon_ntff_profile_hook = lambda h: state.__setitem__("hook", h)
            mod.get_axon_ntff_profile_hook = lambda: state["hook"]
            sys.modules["antenv.axon_hooks"] = mod
            antenv.axon_hooks = mod
            mod.set_axon_ntff_profile_hook(
                _ntff_profile_via_ctypes("/opt/axon/libaxon_pjrt.so")
            )
        # artifact upload needs egress; fall back to the local dir
        from concourse import bass_utils as _bu

        _orig_upload = _bu.upload_artifacts

        def _safe_upload(tmpdir):
            try:
                return _orig_upload(tmpdir)
            except Exception:
                return f"local:{tmpdir}"

        _bu.upload_artifacts = _safe_upload
    except Exception:
        pass


_ensure_trace_support()

B, S, D, K = 32, 2048, 1024, 1024
NCORES = 8
LB = B // NCORES          # local batches per core
P = 128                   # SBUF partitions
NCH = D // P              # contraction chunks (8)
NM = K // P               # output k-blocks (8)
SBLK = 512                # s-tile width (one PSUM bank of fp32)

BF16 = mybir.dt.bfloat16
F32 = mybir.dt.float32
Tanh = mybir.ActivationFunctionType.Tanh


def build(lb=LB, s=S, d=D, k=K, sblk=SBLK, n_warm=42):
    nch = d // P
    nm = k // P
    nsblk = s // sblk

    nc = bacc.Bacc("TRN2", target_bir_lowering=False, debug=False)
    xt_d = nc.declare_dram_parameter("xt", [lb, d, s], BF16, isOutput=False)
    wot_d = nc.declare_dram_parameter("wot", [d, k], BF16, isOutput=False)
    wgt_d = nc.declare_dram_parameter("wgt", [d, k], BF16, isOutput=False)
    gt_d = nc.declare_dram_parameter("gt", [d, lb], BF16, isOutput=False)
    v_d = nc.declare_dram_parameter("v", [k], F32, isOutput=False)
    out_d = nc.declare_dram_parameter("out", [lb, s], F32, isOutput=True)

    with ExitStack() as ctx:
        tc = ctx.enter_context(tile.TileContext(nc))
        const = ctx.enter_context(tc.tile_pool(name="const", bufs=1))
        xpool = ctx.enter_context(tc.tile_pool(name="xpool", bufs=3))
        tpool = ctx.enter_context(tc.tile_pool(name="tpool", bufs=4))
        apool = ctx.enter_context(tc.tile_pool(name="apool", bufs=3))
        opool = ctx.enter_context(tc.tile_pool(name="opool", bufs=2))
        ppool = ctx.enter_context(tc.tile_pool(name="ppool", bufs=4, space="PSUM"))
        popool = ctx.enter_context(tc.tile_pool(name="popool", bufs=3, space="PSUM"))

        # ---- PE warm-up: dummy matmuls fed from a memset tile (no DMA dep)
        # keep the PE busy from t~0 so HAM un-throttles before real work ----
        # zeros: HAM counts instruction activity, not data toggling, and
        # zero operands add no switching power toward the P0 thermal limit
        warm_sb = const.tile([P, P + sblk], BF16)
        nc.vector.memset(warm_sb[:], 0.0)
        ps_w = ppool.tile([P, sblk], F32, tag="pst2", bufs=1)
        for _ in range(n_warm):
            nc.tensor.matmul(
                ps_w[:], warm_sb[:, 0:P], warm_sb[:, P:P + sblk],
                start=True, stop=True,
            )

        # ---- constants. Weights split in k-halves so the first half of
        # term2 and the first m-groups of block 0 start after ~3 MB of DMA;
        # the second halves land during block-0 compute. ----
        splits = 2  # k-halves: quarter-splits measured slower (512-B DMA segments)
        kq = k // splits
        mq = nm // splits
        g_sb = const.tile([P, nch, lb], BF16)
        nc.sync.dma_start(g_sb[:], gt_d.rearrange("(c p) b -> p c b", p=P))
        v_sb = const.tile([P, nm], F32)
        nc.sync.dma_start(v_sb[:], v_d.rearrange("(m p) -> p m", p=P))
        ones_sb = const.tile([P, 1], BF16)
        nc.vector.memset(ones_sb[:], 1.0)
        vb_sb = const.tile([P, nm], BF16)
        nc.vector.tensor_copy(vb_sb[:], v_sb[:])

        wg_sb = const.tile([P, nch, k], BF16)
        wgt_src = wgt_d.rearrange("(c p) k -> p c k", p=P)
        nc.sync.dma_start(wg_sb[:, :, 0:kq], wgt_src[:, :, 0:kq])

        # WO first k-half + first x tile interleaved per-chunk on the sync
        # queue; remaining halves stream while block 0 computes.
        wo_sb = const.tile([P, nch, k], BF16)
        xt0_sb = xpool.tile([P, nch, sblk], BF16, tag="xt")
        xt0_src = xt_d[0].rearrange("(c p) s -> p c s", p=P)[:, :, 0:sblk]
        wot_src = wot_d.rearrange("(c p) k -> p c k", p=P)
        for c in range(nch):
            nc.sync.dma_start(wo_sb[:, c, 0:kq], wot_src[:, c, 0:kq])
            nc.sync.dma_start(xt0_sb[:, c, :], xt0_src[:, c, :])

        def emit_weights_q(q):
            # wg first: term2's next slice consumes it earlier than the main
            # stream reaches the matching wo k-range
            ksl = slice(q * kq, (q + 1) * kq)
            nc.sync.dma_start(wg_sb[:, :, ksl], wgt_src[:, :, ksl])
            for c in range(nch):
                nc.sync.dma_start(wo_sb[:, c, ksl], wot_src[:, c, ksl])

        # term2[k, b] for all local batches: [128, nm * lb] fp32
        term2_sb = const.tile([P, nm * lb], F32)

        def emit_term2(m_lo, m_hi):
            for m in range(m_lo, m_hi):
                ps_t2 = ppool.tile([P, lb], F32, tag="pst2", bufs=1)
                for c in range(nch):
                    nc.tensor.matmul(
                        ps_t2[:],
                        wg_sb[:, c, m * P:(m + 1) * P],
                        g_sb[:, c, :],
                        start=(c == 0),
                        stop=(c == nch - 1),
                    )
                nc.vector.tensor_copy(term2_sb[:, m * lb:(m + 1) * lb], ps_t2[:])

        emit_term2(0, mq)

        # ---- main loop ----
        # xt tiles prefetched one block ahead on the sync queue, which carries
        # nothing else steady-state (outputs go via SWDGE).
        xt_tiles = {(0, 0): xt0_sb}

        def fetch_xt(b, i):
            if (b, i) in xt_tiles or b >= lb or i >= nsblk:
                return
            t = xpool.tile([P, nch, sblk], BF16, tag="xt", name=f"xt_{b}_{i}")
            nc.sync.dma_start(
                t[:],
                xt_d[b].rearrange("(c p) s -> p c s", p=P)
                [:, :, i * sblk:(i + 1) * sblk],
            )
            xt_tiles[(b, i)] = t

        emit_weights_q(1)
        fetch_xt(0, 1)
        for q in range(2, splits):
            emit_weights_q(q)
        for b in range(lb):
            orow = opool.tile([1, s], F32, tag="orow")
            for i in range(nsblk):
                s0 = i * sblk
                nxt = (b, i + 1) if i + 1 < nsblk else (b + 1, 0)
                fetch_xt(*nxt)
                xt_sb = xt_tiles.pop((b, i))
                acc = apool.tile([P, sblk], F32, tag="acc")
                accb = apool.tile([P, sblk], BF16, tag="accb")
                tmp = apool.tile([P, sblk], F32, tag="tmp")
                # On the very last s-block, the last k-block bypasses the DVE
                # chain (its v-weighting folds into the reduce matmul below),
                # shortening the end-of-kernel serial tail by ~1 us.
                last_blk = (b == lb - 1 and i == nsblk - 1)
                dve_last = nm - 2 if last_blk else nm - 1
                th_last = None
                for m in range(nm):
                    if b == 0 and i == 0 and m > 0 and m % mq == 0:
                        # next slice of term2: its wg k-quarter has landed
                        emit_term2(m, m + mq)
                    ps1 = ppool.tile([P, sblk], F32, tag="ps1")
                    for c in range(nch):
                        nc.tensor.matmul(
                            ps1[:],
                            wo_sb[:, c, m * P:(m + 1) * P],
                            xt_sb[:, c, :],
                            start=(c == 0),
                            stop=(c == nch - 1),
                        )
                    th = tpool.tile([P, sblk], BF16, tag="th")
                    nc.scalar.activation(
                        th[:], ps1[:], Tanh,
                        bias=term2_sb[:, m * lb + b:m * lb + b + 1],
                    )
                    # v-weighted accumulate over k-blocks on DVE; the last
                    # DVE op rounds to bf16 so the partition-reduce matmul
                    # below is single-pass bf16 (fp32 PE is 2-pass)
                    if last_blk and m == nm - 1:
                        th_last = th
                    elif m == 0:
                        dst = accb if dve_last == 0 else acc
                        nc.vector.tensor_scalar_mul(dst[:], th[:], v_sb[:, 0:1])
                    else:
                        nc.vector.tensor_scalar_mul(tmp[:], th[:], v_sb[:, m:m + 1])
                        dst = accb if m == dve_last else acc
                        nc.vector.tensor_add(out=dst[:], in0=acc[:], in1=tmp[:])
                # partition reduction: out_row[s] = sum_p accb[p, s]
                ps_o = popool.tile([1, sblk], F32, tag="pso")
                nc.tensor.matmul(
                    ps_o[:], ones_sb[:], accb[:],
                    start=True, stop=not last_blk,
                )
                if last_blk:
                    nc.tensor.matmul(
                        ps_o[:], vb_sb[:, nm - 1:nm], th_last[:],
                        start=False, stop=True,
                    )
                nc.vector.tensor_copy(orow[0:1, s0:s0 + sblk], ps_o[:])
            nc.gpsimd.dma_start(out_d[b:b + 1, :], orow[:])
    nc.compile()
    return nc


def pack_inputs(x, g, WO, WG, v, lb, s=S, d=D, k=K, sblk=SBLK):
    """Pack one core's inputs into the DRAM layouts declared in build()."""
    bf16 = ml_dtypes.bfloat16
    xt = np.ascontiguousarray(x.transpose(0, 2, 1)).astype(bf16)        # [lb, d, s]
    wot = np.ascontiguousarray(WO.T).astype(bf16)                       # [d, k]
    wgt = np.ascontiguousarray(WG.T).astype(bf16)                       # [d, k]
    gt = np.ascontiguousarray(g.T).astype(bf16)                         # [d, lb]
    vi = np.ascontiguousarray(v).astype(np.float32)                     # [k]
    return {"xt": xt, "wot": wot, "wgt": wgt, "gt": gt, "v": vi}


_built = None


def _get_built():
    global _built
    if _built is None:
        _built = build()
    return _built


def make_in_maps(inputs_np):
    x = np.asarray(inputs_np["inputs"], dtype=np.float32)
    g = np.asarray(inputs_np["g"], dtype=np.float32)
    WO = np.asarray(inputs_np["WO"], dtype=np.float32)
    WG = np.asarray(inputs_np["WG"], dtype=np.float32)
    v = np.asarray(inputs_np["v"], dtype=np.float32)[0]

    shared = None
    in_maps = []
    for i in range(NCORES):
        m = pack_inputs(x[i * LB:(i + 1) * LB], g[i * LB:(i + 1) * LB],
                        WO, WG, v, lb=LB)
        if shared is None:
            shared = {kk: m[kk] for kk in ("wot", "wgt", "v")}
        else:
            m.update(shared)  # identical weight images for every core
        in_maps.append(m)
    return in_maps


def run(inputs_np, trace=False):
    nc = _get_built()
    in_maps = make_in_maps(inputs_np)
    res = run_bass_kernel_spmd(nc, in_maps, core_ids=list(range(NCORES)), trace=trace)
    out = np.concatenate(
        [np.asarray(res.results[i]["out"]) for i in range(NCORES)], axis=0
    ).astype(np.float32)
    return out, res


def kernel(**inputs):
    out, _ = run(inputs, trace=False)
    return out



# revision 15
# speedup vs baseline: 1.3602x; 1.3602x over previous
"""Trainium2 Bass kernel for nn_Attention_609885356930.

Reference math (per batch b, sequence s):
    term1[b,s,k] = sum_d WO[k,d] * x[b,s,d]          # big matmul
    term2[b,k]   = sum_d WG[k,d] * g[b,d]            # tiny matmul
    out[b,s]     = sum_k v[k] * tanh(term1 + term2)

Strategy (8 NeuronCores, data-parallel over batch, 4 batches/core):
  - Host pre-transposes x -> xT[b, d, s] and weights (contraction dim d on
    SBUF partitions, no on-device transpose) and sorts the k axis by |v[k]|
    (out is invariant to a joint permutation of WO/WG rows and v).
  - Mixed precision by |v|: term1 errors enter the output through
    v[k]*tanh'(..), so k-blocks with small |v| tolerate coarse math. The
    NF8 lowest-|v| k-blocks (of 8) use fp8 e4m3 matmuls in DoubleRow perf
    mode (256-deep contraction per instruction -> half the PE instructions);
    the high-|v| blocks stay bf16. Measured on HW: fp8-DR and bf16 matmul
    instructions retire at the same 216 ns cadence (512 moving columns at
    ~2.37 GHz, LDWEIGHTS hidden), so PE time scales with instruction count:
    44/64 of the bf16-only kernel. Predicted end-to-end rel-err ~1.3e-2
    (budget 2e-2); fp8 operands are pre-scaled by powers of 2 (x*8, WO*32)
    to dodge e4m3 subnormals, descaled exactly in the ACT pass.
  - term1 computed transposed on-chip: T1[k_block, s] so that
      * term2 becomes a per-partition bias fused into the ACT tanh pass
        (scale=1/256 for fp8 blocks folds the descale in for free)
      * the v-weighted reduce over k runs on the DVE as one fused
        scalar_tensor_tensor (acc' = th*v_m + acc) per k-block in fp16
        (2x DVE rate), finished by a single ones-vector PE matmul per
        s-block (partition reduction).
  - Output DMA per s-block (2 KB) on the gpsimd SWDGE queue to keep the
    end-of-kernel serial tail short.
  - Startup: memset-fed dummy matmuls warm the PE (HAM un-throttle) with no
    DMA dependency; DMA order puts block-(0,0) fp8 operands + the sorted
    WG slices for the fp8 k-blocks first so compute starts after ~2.4 MB.
"""

import sys
import types
import numpy as np
import ml_dtypes
from contextlib import ExitStack

import concourse.bass as bass
import concourse.mybir as mybir
import concourse.tile as tile
from concourse import bacc
from concourse.bass_utils import run_bass_kernel_spmd


def _ensure_trace_support():
    """Make run_bass_kernel_spmd(trace=True) (or BASS_TRACE=1) work under
    axon even when the image's antenv lacks the axon_hooks module; degrade
    silently if anything is missing."""
    try:
        try:
            from antenv.axon_hooks import get_axon_ntff_profile_hook  # noqa: F401
        except ImportError:
            import antenv
            from trn_agent_boot.trn_boot import _ntff_profile_via_ctypes

            mod = types.ModuleType("antenv.axon_hooks")
            state = {"hook": None}
            mod.set_axon_ntff_profile_hook = lambda h: state.__setitem__("hook", h)
            mod.get_axon_ntff_profile_hook = lambda: state["hook"]
            sys.modules["antenv.axon_hooks"] = mod
            antenv.axon_hooks = mod
            mod.set_axon_ntff_profile_hook(
                _ntff_profile_via_ctypes("/opt/axon/libaxon_pjrt.so")
            )
        # artifact upload needs egress; fall back to the local dir
        from concourse import bass_utils as _bu

        _orig_upload = _bu.upload_artifacts

        def _safe_upload(tmpdir):
            try:
                return _orig_upload(tmpdir)
            except Exception:
                return f"local:{tmpdir}"

        _bu.upload_artifacts = _safe_upload
    except Exception:
        pass


_ensure_trace_support()

B, S, D, K = 32, 2048, 1024, 1024
NCORES = 8
LB = B // NCORES          # local batches per core
P = 128                   # SBUF partitions
NCH = D // P              # contraction chunks (8)
NM = K // P               # output k-blocks (8)
NF8 = 5                   # k-blocks (lowest |v|) computed in fp8 DoubleRow
SBLK = 512                # s-tile width (one PSUM bank of fp32)
SX = 8.0                  # fp8 pre-scale on x
SW = 32.0                 # fp8 pre-scale on WO
DS = 1.0 / (SX * SW)      # exact descale folded into ACT

BF16 = mybir.dt.bfloat16
FP16 = mybir.dt.float16
FP8 = mybir.dt.float8e4
F32 = mybir.dt.float32
Tanh = mybir.ActivationFunctionType.Tanh
DR = mybir.MatmulPerfMode.DoubleRow
MULT = mybir.AluOpType.mult
ADD = mybir.AluOpType.add


def build(lb=LB, s=S, d=D, k=K, sblk=SBLK, nf8=NF8, n_warm=30):
    nch = d // P
    nm = k // P
    nbf = nm - nf8
    kf8 = nf8 * P
    kbf = nbf * P
    nsblk = s // sblk

    nc = bacc.Bacc("TRN2", target_bir_lowering=False, debug=False)
    x8_d = nc.declare_dram_parameter("x8", [lb, d, s], FP8, isOutput=False)
    xb_d = nc.declare_dram_parameter("xb", [lb, d, s], BF16, isOutput=False)
    wo8_d = nc.declare_dram_parameter("wo8", [d, kf8], FP8, isOutput=False)
    wob_d = nc.declare_dram_parameter("wob", [d, kbf], BF16, isOutput=False)
    wgt_d = nc.declare_dram_parameter("wgt", [d, k], BF16, isOutput=False)
    gt_d = nc.declare_dram_parameter("gt", [d, lb], BF16, isOutput=False)
    v_d = nc.declare_dram_parameter("v", [k], F32, isOutput=False)
    out_d = nc.declare_dram_parameter("out", [lb, s], F32, isOutput=True)

    with ExitStack() as ctx:
        tc = ctx.enter_context(tile.TileContext(nc))
        const = ctx.enter_context(tc.tile_pool(name="const", bufs=1))
        xpool = ctx.enter_context(tc.tile_pool(name="xpool", bufs=3))
        tpool = ctx.enter_context(tc.tile_pool(name="tpool", bufs=4))
        apool = ctx.enter_context(tc.tile_pool(name="apool", bufs=3))
        opool = ctx.enter_context(tc.tile_pool(name="opool", bufs=3))
        ppool = ctx.enter_context(tc.tile_pool(name="ppool", bufs=4, space="PSUM"))
        popool = ctx.enter_context(tc.tile_pool(name="popool", bufs=3, space="PSUM"))

        # ---- PE warm-up: dummy matmuls fed from a memset tile (no DMA dep)
        # keep the PE busy from t~0 so HAM un-throttles before real work ----
        warm_sb = const.tile([P, P + sblk], BF16)
        nc.vector.memset(warm_sb[:], 0.0)
        ps_w = ppool.tile([P, sblk], F32, tag="pst2", bufs=1)
        for _ in range(n_warm):
            nc.tensor.matmul(
                ps_w[:], warm_sb[:, 0:P], warm_sb[:, P:P + sblk],
                start=True, stop=True,
            )

        # ---- constants / weights. DMA order = first-need order for s-block
        # (0,0): fp8 weights + x8 tile + WG slices for the fp8 blocks, then
        # the bf16 side, then the rest of WG. ----
        g_sb = const.tile([P, nch, lb], BF16)
        nc.sync.dma_start(g_sb[:], gt_d.rearrange("(c p) b -> p c b", p=P))
        v_sb = const.tile([P, nm], F32)
        nc.sync.dma_start(v_sb[:], v_d.rearrange("(m p) -> p m", p=P))
        ones_sb = const.tile([P, 1], FP16)
        nc.vector.memset(ones_sb[:], 1.0)

        wg_sb = const.tile([P, nch, k], BF16)
        wgt_src = wgt_d.rearrange("(c p) k -> p c k", p=P)

        def fetch_wg(k_lo, k_hi):
            nc.sync.dma_start(wg_sb[:, :, k_lo:k_hi], wgt_src[:, :, k_lo:k_hi])

        # DMA order = first-need order: wg first k-half (term2 m0 runs
        # during warmup), first fp8 x tile, fp8 weights; the rest lands
        # under block-(0,0) compute.
        fetch_wg(0, k // 2)

        xt8_tiles = {}
        xtb_tiles = {}

        def fetch_x8(bi, i):
            if (bi, i) in xt8_tiles or bi >= lb or i >= nsblk:
                return
            t = xpool.tile([P, nch, sblk], FP8, tag="xt8", name=f"xt8_{bi}_{i}")
            nc.sync.dma_start(
                t[:],
                x8_d[bi].rearrange("(c p) s -> p c s", p=P)
                [:, :, i * sblk:(i + 1) * sblk],
            )
            xt8_tiles[(bi, i)] = t

        def fetch_xb(bi, i):
            if (bi, i) in xtb_tiles or bi >= lb or i >= nsblk:
                return
            t = xpool.tile([P, nch, sblk], BF16, tag="xtb", name=f"xtb_{bi}_{i}")
            nc.sync.dma_start(
                t[:],
                xb_d[bi].rearrange("(c p) s -> p c s", p=P)
                [:, :, i * sblk:(i + 1) * sblk],
            )
            xtb_tiles[(bi, i)] = t

        fetch_x8(0, 0)
        wo8_sb = const.tile([P, nch, kf8], FP8)
        nc.sync.dma_start(wo8_sb[:], wo8_d.rearrange("(c p) k -> p c k", p=P))
        wob_sb = const.tile([P, nch, kbf], BF16)
        nc.sync.dma_start(wob_sb[:], wob_d.rearrange("(c p) k -> p c k", p=P))
        fetch_wg(k // 2, k)
        fetch_xb(0, 0)

        # term2[k, b] for all local batches: [128, nm * lb] fp32
        term2_sb = const.tile([P, nm * lb], F32)

        def emit_term2(m_lo, m_hi):
            for m in range(m_lo, m_hi):
                ps_t2 = ppool.tile([P, lb], F32, tag="pst2", bufs=1)
                for c in range(nch):
                    nc.tensor.matmul(
                        ps_t2[:],
                        wg_sb[:, c, m * P:(m + 1) * P],
                        g_sb[:, c, :],
                        start=(c == 0),
                        stop=(c == nch - 1),
                    )
                nc.vector.tensor_copy(term2_sb[:, m * lb:(m + 1) * lb], ps_t2[:])

        # ---- main loop ----
        for b in range(lb):
            for i in range(nsblk):
                s0 = i * sblk
                first_blk = (b == 0 and i == 0)
                if first_blk:
                    emit_term2(0, 1)  # m0 bias: runs while x8(0,0) streams
                nxt = (b, i + 1) if i + 1 < nsblk else (b + 1, 0)
                fetch_x8(*nxt)
                fetch_xb(*nxt)
                xt8_sb = xt8_tiles.pop((b, i))
                xtb_sb = xtb_tiles.pop((b, i))
                acc_a = apool.tile([P, sblk], FP16, tag="accA", name="acc_a")
                acc_b = apool.tile([P, sblk], FP16, tag="accB", name="acc_b")
                acc = [acc_a, acc_b]
                for m in range(nm):
                    ps1 = ppool.tile([P, sblk], F32, tag="ps1")
                    if m < nf8:
                        for j in range(nch // 2):
                            nc.tensor.matmul(
                                ps1[:],
                                wo8_sb[:, 2 * j:2 * j + 2, m * P:(m + 1) * P],
                                xt8_sb[:, 2 * j:2 * j + 2, :],
                                start=(j == 0),
                                stop=(j == nch // 2 - 1),
                                perf_mode=DR,
                            )
                    else:
                        for c in range(nch):
                            nc.tensor.matmul(
                                ps1[:],
                                wob_sb[:, c, (m - nf8) * P:(m - nf8 + 1) * P],
                                xtb_sb[:, c, :],
                                start=(c == 0),
                                stop=(c == nch - 1),
                            )
                    if first_blk and m + 1 < nm:
                        # next bias slice: PE computes it between this
                        # block's matmul groups, ahead of its ACT use
                        emit_term2(m + 1, m + 2)
                    th = tpool.tile([P, sblk], FP16, tag="th")
                    nc.scalar.activation(
                        th[:], ps1[:], Tanh,
                        bias=term2_sb[:, m * lb + b:m * lb + b + 1],
                        scale=DS if m < nf8 else 1.0,
                    )
                    # v-weighted accumulate over k-blocks on DVE (fp16, one
                    # fused op per block): acc' = th * v_m + acc
                    if m == 0:
                        nc.vector.tensor_scalar_mul(
                            acc[0][:], th[:], v_sb[:, 0:1])
                    else:
                        nc.vector.scalar_tensor_tensor(
                            acc[m % 2][:], th[:], v_sb[:, m:m + 1],
                            acc[(m - 1) % 2][:], op0=MULT, op1=ADD,
                        )
                # partition reduction: out_row[s] = sum_p acc[p, s]
                ps_o = popool.tile([1, sblk], F32, tag="pso")
                nc.tensor.matmul(
                    ps_o[:], ones_sb[:], acc[(nm - 1) % 2][:],
                    start=True, stop=True,
                )
                oseg = opool.tile([1, sblk], F32, tag="oseg")
                nc.vector.tensor_copy(oseg[:], ps_o[:])
                nc.gpsimd.dma_start(out_d[b:b + 1, s0:s0 + sblk], oseg[:])
    nc.compile()
    return nc


def pack_inputs(x, g, WOs, WGs, vs, lb, d=D, k=K, nf8=NF8):
    """Pack one core's inputs into the DRAM layouts declared in build().
    WOs/WGs/vs are already |v|-sorted along k."""
    bf16 = ml_dtypes.bfloat16
    f8 = ml_dtypes.float8_e4m3
    kf8 = nf8 * P
    xt = np.ascontiguousarray(x.transpose(0, 2, 1))                  # [lb, d, s]
    x8 = (xt * SX).astype(f8)
    xb = xt.astype(bf16)
    wo8 = np.ascontiguousarray(WOs[:kf8].T * SW).astype(f8)          # [d, kf8]
    wob = np.ascontiguousarray(WOs[kf8:].T).astype(bf16)             # [d, kbf]
    wgt = np.ascontiguousarray(WGs.T).astype(bf16)                   # [d, k]
    gt = np.ascontiguousarray(g.T).astype(bf16)                      # [d, lb]
    vi = np.ascontiguousarray(vs).astype(np.float32)                 # [k]
    return {"x8": x8, "xb": xb, "wo8": wo8, "wob": wob, "wgt": wgt,
            "gt": gt, "v": vi}


_built = None


def _get_built():
    global _built
    if _built is None:
        _built = build()
    return _built


def make_in_maps(inputs_np):
    x = np.asarray(inputs_np["inputs"], dtype=np.float32)
    g = np.asarray(inputs_np["g"], dtype=np.float32)
    WO = np.asarray(inputs_np["WO"], dtype=np.float32)
    WG = np.asarray(inputs_np["WG"], dtype=np.float32)
    v = np.asarray(inputs_np["v"], dtype=np.float32)[0]

    order = np.argsort(np.abs(v), kind="stable")
    WOs = np.ascontiguousarray(WO[order])
    WGs = np.ascontiguousarray(WG[order])
    vs = np.ascontiguousarray(v[order])

    shared = None
    in_maps = []
    for i in range(NCORES):
        m = pack_inputs(x[i * LB:(i + 1) * LB], g[i * LB:(i + 1) * LB],
                        WOs, WGs, vs, lb=LB)
        if shared is None:
            shared = {kk: m[kk] for kk in ("wo8", "wob", "wgt", "v")}
        else:
            m.update(shared)  # identical weight images for every core
        in_maps.append(m)
    return in_maps


def run(inputs_np, trace=False):
    nc = _get_built()
    in_maps = make_in_maps(inputs_np)
    res = run_bass_kernel_spmd(nc, in_maps, core_ids=list(range(NCORES)), trace=trace)
    out = np.concatenate(
        [np.asarray(res.results[i]["out"]) for i in range(NCORES)], axis=0
    ).astype(np.float32)
    return out, res


def kernel(**inputs):
    out, _ = run(inputs, trace=False)
    return out
